# revision 18
# baseline (speedup 1.0000x reference)
"""Trainium2 Bass kernel for nn_HardwiredAttention (NRI-style GNN message passing).

Math (derived from the reference):
  adj[b,t,i,j] = 1/(||locs[b,i,t]-locs[b,j,t]|| + eps) for i!=j, 0 on diag
  out[b,:,t,:] = adj[b,t] @ hidden[b,:,t,:]          ([48,48] @ [48,128] per (b,t))

The rel_rec/rel_send one-hot matmuls in the reference are just gathers over the
fixed fully-connected off-diagonal edge pattern; adj is symmetric.

Distribution: data-parallel over batch, 2 batches per core, 8 cores, no comms.

Per-core layout:
  - elementwise pipeline in partitions p=(s,tau), t=2*tau+s (100 partitions):
    dx/dy from a tiny [100,(c,b,n)] coords tile via stride-0 broadcast APs,
    squares on ACT, d2-add on GPSIMD, sqrt on ACT, (s+eps)+BIGMASK via
    scalar_tensor_tensor, reciprocal_approx_fast on DVE, fp16 convert on ACT.
  - PE transposes [50(tau),48(j)] -> [48(j),50(tau)] per (b,i,s) build a
    block-diagonal fp16 lhsT [96=(s,j), (b,scol,i,tau)].
  - 2-packed matmuls lhsT[96,96] @ hidden[96,128] -> PSUM [96,128] fp32,
    DMA'd straight to HBM.
"""

import os
import sys

sys.path.insert(0, "/opt/trn_rl_repo")

import numpy as np

import bass_rust
import concourse.bass as bass
import concourse.tile as tile
from concourse import bacc, mybir
from concourse.bass_utils import run_bass_kernel_spmd

F32 = mybir.dt.float32
F16 = mybir.dt.float16
ALU = mybir.AluOpType

B, N, T, H = 16, 48, 100, 128
NCORES = 8
BL = B // NCORES          # 2 batches per core
TAU = T // 2              # 50
E = N * N                 # 2304 (full pair matrix incl. diag)
EPS = 1e-5
BIG = 60000.0             # diag mask: 1/(BIG) ~ 1.7e-5 ~ 0 in fp16
GI = 8                    # i's per PSUM transpose group


def _ap(t, offset, dims):
    """Manual access pattern on a tile/tensor handle's underlying tensor."""
    return bass_rust.AP(t.tensor, offset, [list(d) for d in dims])


def _build_nc_bl(bl):
    """Build the device kernel for a given batches-per-core (module BL swap)."""
    global BL
    old = BL
    BL = bl
    try:
        return build_nc()
    finally:
        BL = old


def build_nc():
    nc = bacc.Bacc("TRN2", target_bir_lowering=False, debug=False)

    # DRAM I/O (per core). Layouts chosen so every DMA is long-contiguous.
    xt = nc.dram_tensor("xt", [2, 128, BL * N], F32, kind="ExternalInput")
    hid = nc.dram_tensor("hid", [BL, 2, N, TAU, H], F16, kind="ExternalInput")
    bm = nc.dram_tensor("bm", [128, E], F16, kind="ExternalInput")
    ident = nc.dram_tensor("ident", [128, TAU], F16, kind="ExternalInput")
    out = nc.dram_tensor("out", [BL, 2, N, TAU, H], F16, kind="ExternalOutput")

    with tile.TileContext(nc) as tc:
        _emit(nc, tc, xt, hid, bm, ident, out)
    nc.compile()
    return nc


def _emit(nc, tc, xt, hid, bm, ident, out):
    FREE = BL * E  # 4608 free elems/partition for pair tiles

    with (
        tc.tile_pool(name="persist", bufs=1) as pp,
        tc.tile_pool(name="tp", bufs=3, space="PSUM") as tp_pool,
        tc.tile_pool(name="mm", bufs=4, space="PSUM") as mm_pool,
        tc.tile_pool(name="ot", bufs=6) as ot_pool,
    ):
        xt_sb = pp.tile([128, 2 * BL * N], F32, tag="xt")
        hid_sb = pp.tile([128, BL * TAU * H], F16, tag="hid")
        bm_sb = pp.tile([128, E], F16, tag="bm")
        id_sb = pp.tile([128, TAU], F16, tag="id")
        dx = pp.tile([128, FREE], F32, tag="dx")
        dy = pp.tile([128, FREE], F32, tag="dy")
        dx2 = pp.tile([128, FREE], F32, tag="dx2")
        dy2 = pp.tile([128, FREE], F32, tag="dy2")
        adj16 = pp.tile([128, FREE], F16, tag="adj16")
        lhsT = pp.tile([128, BL * 2 * N * TAU], F16, tag="lhsT")

        # ---- loads -------------------------------------------------------
        nc.sync.dma_start(xt_sb[:], xt.ap().rearrange("c p q -> p c q"))
        nc.vector.memset(hid_sb[:], 0.0)
        for s in range(2):
            nc.sync.dma_start(
                hid_sb[s * 64 : s * 64 + N, :],
                hid[:, s].rearrange("b j t h -> j b t h"),
            )
        nc.sync.dma_start(bm_sb[:], bm.ap())
        nc.sync.dma_start(id_sb[:], ident.ap())
        # zero the off-diagonal blocks of the block-diag lhsT (whole tensor)
        nc.vector.memset(lhsT[:], 0.0)

        # ---- pairwise distance chain ------------------------------------
        # xt_sb free layout: (c, b, n); strides c:96, b:48, n:1
        def coords_ap(c, vary_i):
            base = c * (BL * N)
            if vary_i:
                dims = [[2 * BL * N, 128], [N, BL], [1, N], [0, N]]
            else:
                dims = [[2 * BL * N, 128], [N, BL], [0, N], [1, N]]
            return _ap(xt_sb[:], base, dims)

        def pair_view(tl):
            return _ap(tl[:], 0, [[FREE, 128], [E, BL], [N, N], [1, N]])

        # chunked over (b, i-half) so the engine chain pipelines
        LFREE = BL * 2 * N * TAU  # 9600
        IH = N // 2               # 24 i's per chunk
        CH = IH * N               # 1152 free elems per chunk
        for b in range(BL):
            for ih in range(2):
                i0 = ih * IH
                off = b * E + i0 * N
                pv = lambda tl: _ap(tl[:], off, [[FREE, 128], [N, IH], [1, N]])
                fl = lambda tl: _ap(tl[:], off, [[FREE, 128], [1, CH]])
                cb = lambda c, vi: _ap(
                    xt_sb[:], c * (BL * N) + b * N + (i0 if vi else 0),
                    [[2 * BL * N, 128], [1, IH], [0, N]] if vi
                    else [[2 * BL * N, 128], [0, IH], [1, N]],
                )
                nc.vector.tensor_tensor(pv(dx), cb(0, True), cb(0, False), ALU.subtract)
                nc.vector.tensor_tensor(pv(dy), cb(1, True), cb(1, False), ALU.subtract)
                nc.scalar.square(fl(dx2), fl(dx))
                nc.scalar.square(fl(dy2), fl(dy))
                nc.gpsimd.tensor_tensor(fl(dx), fl(dx2), fl(dy2), ALU.add)
                nc.scalar.sqrt(fl(dy), fl(dx))
                bm_ap = _ap(bm_sb[:], i0 * N, [[E, 128], [1, CH]])
                nc.vector.scalar_tensor_tensor(
                    fl(dx2), fl(dy), EPS, bm_ap, ALU.add, ALU.add
                )
                nc.vector.reciprocal_approx_fast(out=fl(dy2), in_=fl(dx2))
                nc.scalar.copy(fl(adj16), fl(dy2))

                for s in range(2):
                    for gl in range(IH // GI):
                        g = i0 // GI + gl
                        pt = tp_pool.tile([48, GI * TAU], F16, tag="tp")
                        for ii in range(GI):
                            i = g * GI + ii
                            src = adj16[s * 64 : s * 64 + TAU,
                                        b * E + i * N : b * E + i * N + N]
                            nc.tensor.transpose(
                                pt[:, ii * TAU : (ii + 1) * TAU], src,
                                id_sb[s * 64 : s * 64 + TAU, :]
                            )
                        dst = _ap(
                            lhsT[:],
                            (s * 64) * LFREE + b * (2 * N * TAU) + s * (N * TAU)
                            + g * GI * TAU,
                            [[LFREE, 48], [TAU, GI], [1, TAU]],
                        )
                        csrc = _ap(pt[:], 0, [[GI * TAU, 48], [TAU, GI], [1, TAU]])
                        if g % 2 == 0:
                            nc.vector.tensor_copy(dst, csrc)
                        else:
                            nc.scalar.copy(dst, csrc)

        # ---- packed matmuls + store -------------------------------------
        HF = BL * TAU * H  # hid_sb free size (12800)
        groups = [(tg * 4, min(4, TAU - tg * 4)) for tg in range((TAU + 3) // 4)]
        for b in range(BL):
            for t0, tlen in groups:
                mt = mm_pool.tile([96, 4 * H], F32, tag="mm")
                for k in range(tlen):
                    tau = t0 + k
                    w_ap = _ap(
                        lhsT[:], b * (2 * N * TAU) + tau, [[LFREE, 128], [TAU, 96]]
                    )
                    r_ap = _ap(
                        hid_sb[:], b * (TAU * H) + tau * H, [[HF, 128], [1, H]]
                    )
                    nc.tensor.matmul(
                        mt[:, k * H : (k + 1) * H], w_ap, r_ap,
                        start=True, stop=True,
                    )
                ot = ot_pool.tile([96, 4 * H], F16, tag="ot")
                if t0 % 8 == 0:
                    nc.scalar.copy(ot[:, : tlen * H], mt[:, : tlen * H])
                else:
                    nc.vector.tensor_copy(ot[:, : tlen * H], mt[:, : tlen * H])
                dst = out[b, :, :, t0 : t0 + tlen, :].rearrange(
                    "s i t h -> (s i) (t h)"
                )
                nc.sync.dma_start(dst, ot[:, : tlen * H])


# ----------------------------------------------------------------------------
# Host side
# ----------------------------------------------------------------------------

try:
    import torch  # SIMD f16<->f32 convert + layout change in one pass

    _HAVE_TORCH = True
except Exception:
    _HAVE_TORCH = False


def _prep_xt_all(locs):
    """locs [16,48,100,2] f32 -> concat-ready xt [16,128,96] (c-major per core)."""
    lc = locs.reshape(NCORES, BL, N, TAU, 2, 2)  # (core, bl, n, tau, s, c)
    tr = lc.transpose(0, 5, 4, 3, 1, 2)          # (core, c, s, tau, bl, n)
    xt = np.zeros((NCORES, 2, 2, 64, BL * N), dtype=np.float32)
    xt[:, :, :, :TAU] = tr.reshape(NCORES, 2, 2, TAU, BL * N)
    # filler rows: spread points (x=n, y=0) so junk weights stay finite
    fill = np.tile(np.arange(N, dtype=np.float32), BL)
    xt[:, 0, :, TAU:] = fill[None, None, None, :]
    return xt.reshape(NCORES * 2, 128, BL * N)


def _prep_hid_all(hidden):
    """hidden [16,48,100,128] f32 -> [16,2,48,50,128] f16 (b, s, j, tau, h)."""
    out = np.empty((B, 2, N, TAU, H), dtype=np.float16)
    if _HAVE_TORCH:
        src = torch.from_numpy(hidden).view(B, N, TAU, 2, H).permute(0, 3, 1, 2, 4)
        torch.from_numpy(out).copy_(src)
    else:
        hc = hidden.astype(np.float16).reshape(B, N, TAU, 2, H)
        out[:] = hc.transpose(0, 3, 1, 2, 4)
    return out


def _post_out_all(parts):
    """8 shards [2,2,48,50,128] f16 (b,s,i,tau,h) -> [16,48,100,128] f32."""
    res = np.empty((B, N, T, H), dtype=np.float32)
    if _HAVE_TORCH:
        dst = torch.from_numpy(res).view(B, N, TAU, 2, H)
        for k, p in parts:
            src = torch.from_numpy(p).permute(0, 2, 3, 1, 4)  # (b,i,tau,s,h)
            dst[BL * k : BL * k + BL].copy_(src)
    else:
        rv = res.reshape(B, N, TAU, 2, H)
        for k, p in parts:
            rv[BL * k : BL * k + BL] = p.astype(np.float32).transpose(0, 2, 3, 1, 4)
    return res


_BM = None
_IDENT = None


def _consts():
    global _BM, _IDENT
    if _BM is None:
        row = (BIG * np.eye(N, dtype=np.float32)).astype(np.float16).reshape(1, E)
        _BM = np.ascontiguousarray(np.repeat(row, 128, axis=0))
        idm = np.zeros((128, TAU), dtype=np.float16)
        idm[0:TAU] = np.eye(TAU, dtype=np.float16)
        idm[64 : 64 + TAU] = np.eye(TAU, dtype=np.float16)
        _IDENT = idm
    return _BM, _IDENT


LAST_EXEC_NS = None


class _FastPath:
    """Cached jit(shard_map(bass_exec)) dispatch — built once, reused per call.

    run_bass_via_pjrt (what run_bass_kernel_spmd delegates to under axon)
    rebuilds the jit wrapper on every call (~0.5s) and ships the consts and a
    zero output-donation buffer over the tunnel each time. This keeps the
    jitted callable, the replicated consts, and a device-resident donation
    buffer alive across calls.
    """

    def __init__(self, nc):
        import jax
        from jax.sharding import Mesh, PartitionSpec, NamedSharding
        from jax.experimental.shard_map import shard_map
        from concourse import bass2jax, mybir as _mybir

        self.jax = jax
        bass2jax.install_neuronx_cc_hook()
        partition_name = (
            nc.partition_id_tensor.name if nc.partition_id_tensor else None
        )
        in_names, out_names, out_avals, zero_outs = [], [], [], []
        for alloc in nc.m.functions[0].allocations:
            if not isinstance(alloc, _mybir.MemoryLocationSet):
                continue
            name = alloc.memorylocations[0].name
            if alloc.kind == "ExternalInput":
                if name != partition_name:
                    in_names.append(name)
            elif alloc.kind == "ExternalOutput":
                shape = tuple(alloc.tensor_shape)
                dtype = _mybir.dt.np(alloc.dtype)
                out_avals.append(jax.core.ShapedArray(shape, dtype))
                out_names.append(name)
                zero_outs.append(np.zeros(shape, dtype))
        n_params = len(in_names)
        in_names_all = list(in_names) + out_names
        if partition_name is not None:
            in_names_all.append(partition_name)
        self.in_names = in_names

        def _body(*args):
            operands = list(args)
            if partition_name is not None:
                operands.append(bass2jax.partition_id_tensor())
            outs = bass2jax._bass_exec_p.bind(
                *operands,
                out_avals=tuple(out_avals),
                in_names=tuple(in_names_all),
                out_names=tuple(out_names),
                lowering_input_output_aliases=(),
                sim_require_finite=True,
                sim_require_nnan=True,
                nc=nc,
            )
            return tuple(outs)

        devices = jax.devices()[:NCORES]
        self.devices = devices
        mesh = Mesh(np.asarray(devices), ("core",))
        n_outs = len(out_avals)
        self.fn = jax.jit(
            shard_map(
                _body,
                mesh=mesh,
                in_specs=(PartitionSpec("core"),) * (n_params + n_outs),
                out_specs=(PartitionSpec("core"),) * n_outs,
                check_rep=False,
            ),
            donate_argnums=tuple(range(n_params, n_params + n_outs)),
            keep_unused=True,
        )
        self.shd = NamedSharding(mesh, PartitionSpec("core"))
        bm, ident = _consts()
        self.bm_dev = jax.device_put(
            np.broadcast_to(bm, (NCORES, *bm.shape)).reshape(NCORES * 128, E),
            self.shd,
        )
        self.ident_dev = jax.device_put(
            np.broadcast_to(ident, (NCORES, *ident.shape)).reshape(NCORES * 128, TAU),
            self.shd,
        )
        # donation seed: contents never read (kernel writes every element)
        self.prev_out = jax.device_put(
            np.zeros((B, 2, N, TAU, H), np.float16), self.shd
        )
        jax.block_until_ready([self.bm_dev, self.ident_dev, self.prev_out])
        import concurrent.futures as _cf

        # 4 workers beat 8 on the single host CPU (less thrash, same overlap)
        self.pool = _cf.ThreadPoolExecutor(4)

        # AOT-compile to skip pjit python dispatch (~40ms/call)
        dummy = {
            "xt": np.zeros((NCORES * 2, 128, BL * N), np.float32),
            "hid": self._put_hid(np.zeros((B, 2, N, TAU, H), np.float16)),
            "bm": self.bm_dev,
            "ident": self.ident_dev,
        }
        self.fn_c = self.fn.lower(
            *[dummy[n] for n in self.in_names], self.prev_out
        ).compile()

    def _put_hid(self, hid_all):
        """Upload the 8 hid shards in parallel threads, assemble zero-copy."""
        jax = self.jax
        futs = [
            self.pool.submit(jax.device_put, hid_all[BL * k : BL * k + BL], d)
            for k, d in enumerate(self.devices)
        ]
        shards = [f.result() for f in futs]
        return jax.make_array_from_single_device_arrays(
            (B, 2, N, TAU, H), self.shd, shards
        )

    def _convert_put_hid(self, hidden):
        """Per-shard f32->f16 layout convert fused with async upload."""
        jax = self.jax

        def _one(k):
            sl = hidden[BL * k : BL * k + BL]
            buf = np.empty((BL, 2, N, TAU, H), dtype=np.float16)
            if _HAVE_TORCH:
                src = torch.from_numpy(sl).view(BL, N, TAU, 2, H).permute(
                    0, 3, 1, 2, 4
                )
                torch.from_numpy(buf).copy_(src)
            else:
                hc = sl.astype(np.float16).reshape(BL, N, TAU, 2, H)
                buf[:] = hc.transpose(0, 3, 1, 2, 4)
            return jax.device_put(buf, self.devices[k])

        futs = [self.pool.submit(_one, k) for k in range(NCORES)]
        shards = [f.result() for f in futs]
        return jax.make_array_from_single_device_arrays(
            (B, 2, N, TAU, H), self.shd, shards
        )

    def run(self, xt_all, hid_g, res):
        """Dispatch one full-batch execution; fill res [B,N,T,H] f32."""
        args = {
            "xt": xt_all,
            "hid": hid_g,
            "bm": self.bm_dev,
            "ident": self.ident_dev,
        }
        outs = self.fn_c(*[args[n] for n in self.in_names], self.prev_out)
        g = outs[0]

        def _fetch_one(shard):
            k = shard.index[0].start // BL
            arr = np.asarray(shard.data)  # [BL,2,N,TAU,H] (b,s,i,tau,h)
            if _HAVE_TORCH:
                dst = torch.from_numpy(res).view(B, N, TAU, 2, H)[
                    BL * k : BL * k + BL
                ]
                dst.copy_(torch.from_numpy(arr).permute(0, 2, 3, 1, 4))
            else:
                res.reshape(B, N, TAU, 2, H)[BL * k : BL * k + BL] = (
                    arr.astype(np.float32).transpose(0, 2, 3, 1, 4)
                )

        shards = sorted(g.addressable_shards, key=lambda s: s.index[0].start)
        list(self.pool.map(_fetch_one, shards))
        self.prev_out = g  # donated next call, after this call's host copies
        return res


def _prep_xt_half(locs_h):
    """locs_h [8,48,100,2] f32 -> xt [16,128,48] for the BL=1 kernel."""
    lc = locs_h.reshape(NCORES, 1, N, TAU, 2, 2)
    tr = lc.transpose(0, 5, 4, 3, 1, 2)
    xt = np.zeros((NCORES, 2, 2, 64, N), dtype=np.float32)
    xt[:, :, :, :TAU] = tr.reshape(NCORES, 2, 2, TAU, N)
    xt[:, 0, :, TAU:] = np.arange(N, dtype=np.float32)[None, None, None, :]
    return xt.reshape(NCORES * 2, 128, N)


class _HalfPath:
    """Two pipelined half-batch (BL=1) executions per call.

    Exec launch is globally gated on all shard inputs per executable, so one
    full-batch call serializes H2D -> exec -> D2H. Splitting into two 8-batch
    launches lets half B's upload/exec overlap half A's execution and D2H.
    """

    def __init__(self, st, nc1):
        import jax
        from jax.sharding import Mesh, PartitionSpec
        from jax.experimental.shard_map import shard_map
        from concourse import bass2jax, mybir as _mybir

        self.jax = jax
        self.st = st
        partition_name = (
            nc1.partition_id_tensor.name if nc1.partition_id_tensor else None
        )
        in_names, out_names, out_avals = [], [], []
        for alloc in nc1.m.functions[0].allocations:
            if not isinstance(alloc, _mybir.MemoryLocationSet):
                continue
            name = alloc.memorylocations[0].name
            if alloc.kind == "ExternalInput":
                if name != partition_name:
                    in_names.append(name)
            elif alloc.kind == "ExternalOutput":
                out_avals.append(
                    jax.core.ShapedArray(
                        tuple(alloc.tensor_shape), _mybir.dt.np(alloc.dtype)
                    )
                )
                out_names.append(name)
        n_params = len(in_names)
        in_names_all = list(in_names) + out_names
        if partition_name is not None:
            in_names_all.append(partition_name)
        self.in_names = in_names

        def _body(*args):
            operands = list(args)
            if partition_name is not None:
                operands.append(bass2jax.partition_id_tensor())
            return tuple(
                bass2jax._bass_exec_p.bind(
                    *operands,
                    out_avals=tuple(out_avals),
                    in_names=tuple(in_names_all),
                    out_names=tuple(out_names),
                    lowering_input_output_aliases=(),
                    sim_require_finite=True,
                    sim_require_nnan=True,
                    nc=nc1,
                )
            )

        mesh = Mesh(np.asarray(st.devices), ("core",))
        fn = jax.jit(
            shard_map(
                _body,
                mesh=mesh,
                in_specs=(PartitionSpec("core"),) * (n_params + 1),
                out_specs=(PartitionSpec("core"),),
                check_rep=False,
            ),
            donate_argnums=(n_params,),
            keep_unused=True,
        )
        self.bufA = jax.device_put(
            np.zeros((NCORES, 2, N, TAU, H), np.float16), st.shd
        )
        self.bufB = jax.device_put(
            np.zeros((NCORES, 2, N, TAU, H), np.float16), st.shd
        )
        jax.block_until_ready([self.bufA, self.bufB])
        dummy = {
            "xt": np.zeros((NCORES * 2, 128, N), np.float32),
            "hid": self._put_hid_half(np.zeros((NCORES, N, T, H), np.float32)),
            "bm": st.bm_dev,
            "ident": st.ident_dev,
        }
        self.fn_c = fn.lower(
            *[dummy[n] for n in in_names], self.bufA
        ).compile()

    def _put_hid_half(self, hid_h):
        """[8,48,100,128] f32 -> sharded [8,2,48,50,128] f16 (convert+upload)."""
        jax, st = self.jax, self.st

        def _one(k):
            buf = np.empty((1, 2, N, TAU, H), dtype=np.float16)
            if _HAVE_TORCH:
                src = torch.from_numpy(hid_h[k : k + 1]).view(
                    1, N, TAU, 2, H
                ).permute(0, 3, 1, 2, 4)
                torch.from_numpy(buf).copy_(src)
            else:
                hc = hid_h[k : k + 1].astype(np.float16).reshape(1, N, TAU, 2, H)
                buf[:] = hc.transpose(0, 3, 1, 2, 4)
            return jax.device_put(buf, st.devices[k])

        futs = [st.pool.submit(_one, k) for k in range(NCORES)]
        shards = [f.result() for f in futs]
        return jax.make_array_from_single_device_arrays(
            (NCORES, 2, N, TAU, H), st.shd, shards
        )

    def _fetch_half(self, g, b0, res):
        def _one(shard):
            k = shard.index[0].start
            arr = np.asarray(shard.data)  # [1,2,N,TAU,H] (b,s,i,tau,h)
            if _HAVE_TORCH:
                dst = torch.from_numpy(res).view(B, N, TAU, 2, H)[
                    b0 + k : b0 + k + 1
                ]
                dst.copy_(torch.from_numpy(arr).permute(0, 2, 3, 1, 4))
            else:
                res.reshape(B, N, TAU, 2, H)[b0 + k : b0 + k + 1] = (
                    arr.astype(np.float32).transpose(0, 2, 3, 1, 4)
                )

        shards = sorted(g.addressable_shards, key=lambda s: s.index[0].start)
        list(self.st.pool.map(_one, shards))

    def run(self, locs, hidden, res):
        st = self.st
        xtA = _prep_xt_half(locs[:NCORES])
        xtB = _prep_xt_half(locs[NCORES:])
        argsA = {
            "xt": xtA,
            "hid": self._put_hid_half(hidden[:NCORES]),
            "bm": st.bm_dev,
            "ident": st.ident_dev,
        }
        gA = self.fn_c(*[argsA[n] for n in self.in_names], self.bufA)[0]
        argsB = {
            "xt": xtB,
            "hid": self._put_hid_half(hidden[NCORES:]),
            "bm": st.bm_dev,
            "ident": st.ident_dev,
        }
        gB = self.fn_c(*[argsB[n] for n in self.in_names], self.bufB)[0]
        self._fetch_half(gA, 0, res)
        self._fetch_half(gB, NCORES, res)
        self.bufA, self.bufB = gA, gB
        return res


_NC = None
_NC1 = None
_FAST = None
_HALF = None


def _get_nc():
    global _NC
    if _NC is None:
        _NC = build_nc()
    return _NC


def _spmd_call(locs, hidden):
    """Robust path: per-call run_bass_kernel_spmd (slower, no cached state)."""
    nc = _get_nc()
    xt_all = _prep_xt_all(locs)
    hid_all = _prep_hid_all(hidden)
    bm, ident = _consts()
    in_maps = [
        {
            "xt": xt_all[2 * k : 2 * k + 2],
            "hid": hid_all[BL * k : BL * k + BL],
            "bm": bm,
            "ident": ident,
        }
        for k in range(NCORES)
    ]
    res = run_bass_kernel_spmd(nc, in_maps, list(range(NCORES)), trace=False)
    global LAST_EXEC_NS
    LAST_EXEC_NS = getattr(res, "exec_time_ns", None)
    parts = [
        (k, res.results[k]["out"].reshape(BL, 2, N, TAU, H)) for k in range(NCORES)
    ]
    return _post_out_all(parts), xt_all, hid_all


def _cold_call(locs, hidden):
    """First call: run via run_bass_kernel_spmd, then build+warm fast paths."""
    global _FAST, _HALF, _NC1
    out, xt_all, hid_all = _spmd_call(locs, hidden)
    # build + fully warm the cached dispatch so call #2 is pure fast path
    _FAST = _FastPath(_get_nc())
    _FAST.run(xt_all, _FAST._put_hid(hid_all), np.empty((B, N, T, H), np.float32))
    try:
        _NC1 = _build_nc_bl(1)
        _HALF = _HalfPath(_FAST, _NC1)
        _HALF.run(locs, hidden, np.empty((B, N, T, H), np.float32))
    except Exception:
        _HALF = None
    return out


def kernel(locs, hidden, rel_rec=None, rel_send=None):
    global _FAST, _HALF
    locs = np.ascontiguousarray(locs, dtype=np.float32)
    hidden = np.ascontiguousarray(hidden, dtype=np.float32)
    if _HAVE_TORCH and not hidden.flags.writeable:
        hidden = hidden.copy()
    if _FAST is None:
        return _cold_call(locs, hidden)
    res = np.empty((B, N, T, H), dtype=np.float32)
    if _HALF is not None:
        try:
            return _HALF.run(locs, hidden, res)
        except Exception:
            _HALF = None  # fall back to the single-launch fast path
    try:
        xt_all = _prep_xt_all(locs)
        hid_g = _FAST._convert_put_hid(hidden)
        return _FAST.run(xt_all, hid_g, res)
    except Exception:
        # fast-path state (donation chain / device session) may be broken;
        # rebuild lazily on the next call and answer via the robust path now
        _FAST = None
        return _spmd_call(locs, hidden)[0]


if __name__ == "__main__":
    # smoke test with random data against a local numpy reference
    rng = np.random.default_rng(0)
    locs = rng.standard_normal((B, N, T, 2), dtype=np.float32)
    hidden = rng.standard_normal((B, N, T, H), dtype=np.float32)
    got = kernel(locs, hidden)
    x = locs[..., 0]
    y = locs[..., 1]
    d = np.sqrt((x[:, :, None] - x[:, None]) ** 2 + (y[:, :, None] - y[:, None]) ** 2)
    w = 1.0 / (d + EPS) * (1.0 - np.eye(N)[None, :, :, None])
    want = np.einsum("bijt,bjth->bith", w.astype(np.float32), hidden)
    err = np.linalg.norm(got - want) / np.linalg.norm(want)
    print("rel err vs numpy:", err)



# revision 20
# speedup vs baseline: 1.0583x; 1.0583x over previous
"""Trainium2 Bass kernel for nn_HardwiredAttention (NRI-style GNN message passing).

Math (derived from the reference):
  adj[b,t,i,j] = 1/(||locs[b,i,t]-locs[b,j,t]|| + eps) for i!=j, 0 on diag
  out[b,:,t,:] = adj[b,t] @ hidden[b,:,t,:]          ([48,48] @ [48,128] per (b,t))

The rel_rec/rel_send one-hot matmuls in the reference are just gathers over the
fixed fully-connected off-diagonal edge pattern; adj is symmetric.

Distribution: data-parallel over batch, 2 batches per core, 8 cores, no comms.

Per-core layout:
  - elementwise pipeline in partitions p=(s,tau), t=2*tau+s (100 partitions):
    dx/dy from a tiny [100,(c,b,n)] coords tile via stride-0 broadcast APs,
    squares on ACT, d2-add on GPSIMD, sqrt on ACT, (s+eps)+BIGMASK via
    scalar_tensor_tensor, reciprocal_approx_fast on DVE, fp16 convert on ACT.
  - PE transposes [50(tau),48(j)] -> [48(j),50(tau)] per (b,i,s) build a
    block-diagonal fp16 lhsT [96=(s,j), (b,scol,i,tau)].
  - 2-packed matmuls lhsT[96,96] @ hidden[96,128] -> PSUM [96,128] fp32,
    DMA'd straight to HBM.
"""

import os
import sys

sys.path.insert(0, "/opt/trn_rl_repo")

import numpy as np

import bass_rust
import concourse.bass as bass
import concourse.tile as tile
from concourse import bacc, mybir
from concourse.bass_utils import run_bass_kernel_spmd

F32 = mybir.dt.float32
F16 = mybir.dt.float16
ALU = mybir.AluOpType

B, N, T, H = 16, 48, 100, 128
NCORES = 8
BL = B // NCORES          # 2 batches per core
TAU = T // 2              # 50
E = N * N                 # 2304 (full pair matrix incl. diag)
EPS = 1e-5
BIG = 60000.0             # diag mask: 1/(BIG) ~ 1.7e-5 ~ 0 in fp16
GI = 8                    # i's per PSUM transpose group


def _ap(t, offset, dims):
    """Manual access pattern on a tile/tensor handle's underlying tensor."""
    return bass_rust.AP(t.tensor, offset, [list(d) for d in dims])


def _build_nc_bl(bl):
    """Build the device kernel for a given batches-per-core (module BL swap)."""
    global BL
    old = BL
    BL = bl
    try:
        return build_nc()
    finally:
        BL = old


def build_nc():
    nc = bacc.Bacc("TRN2", target_bir_lowering=False, debug=False)

    # DRAM I/O (per core). Layouts chosen so every DMA is long-contiguous.
    xt = nc.dram_tensor("xt", [2, 128, BL * N], F32, kind="ExternalInput")
    hid = nc.dram_tensor("hid", [BL, 2, N, TAU, H], F16, kind="ExternalInput")
    bm = nc.dram_tensor("bm", [128, E], F16, kind="ExternalInput")
    ident = nc.dram_tensor("ident", [128, TAU], F16, kind="ExternalInput")
    out = nc.dram_tensor("out", [BL, 2, N, TAU, H], F16, kind="ExternalOutput")

    with tile.TileContext(nc) as tc:
        _emit(nc, tc, xt, hid, bm, ident, out)
    nc.compile()
    return nc


def _emit(nc, tc, xt, hid, bm, ident, out):
    FREE = BL * E  # 4608 free elems/partition for pair tiles

    with (
        tc.tile_pool(name="persist", bufs=1) as pp,
        tc.tile_pool(name="tp", bufs=3, space="PSUM") as tp_pool,
        tc.tile_pool(name="mm", bufs=4, space="PSUM") as mm_pool,
        tc.tile_pool(name="ot", bufs=6) as ot_pool,
    ):
        xt_sb = pp.tile([128, 2 * BL * N], F32, tag="xt")
        hid_sb = pp.tile([128, BL * TAU * H], F16, tag="hid")
        bm_sb = pp.tile([128, E], F16, tag="bm")
        id_sb = pp.tile([128, TAU], F16, tag="id")
        dx = pp.tile([128, FREE], F32, tag="dx")
        dy = pp.tile([128, FREE], F32, tag="dy")
        dx2 = pp.tile([128, FREE], F32, tag="dx2")
        dy2 = pp.tile([128, FREE], F32, tag="dy2")
        adj16 = pp.tile([128, FREE], F16, tag="adj16")
        lhsT = pp.tile([128, BL * 2 * N * TAU], F16, tag="lhsT")

        # ---- loads -------------------------------------------------------
        nc.sync.dma_start(xt_sb[:], xt.ap().rearrange("c p q -> p c q"))
        nc.vector.memset(hid_sb[:], 0.0)
        for s in range(2):
            nc.sync.dma_start(
                hid_sb[s * 64 : s * 64 + N, :],
                hid[:, s].rearrange("b j t h -> j b t h"),
            )
        nc.sync.dma_start(bm_sb[:], bm.ap())
        nc.sync.dma_start(id_sb[:], ident.ap())
        # zero the off-diagonal blocks of the block-diag lhsT (whole tensor)
        nc.vector.memset(lhsT[:], 0.0)

        # ---- pairwise distance chain ------------------------------------
        # xt_sb free layout: (c, b, n); strides c:96, b:48, n:1
        def coords_ap(c, vary_i):
            base = c * (BL * N)
            if vary_i:
                dims = [[2 * BL * N, 128], [N, BL], [1, N], [0, N]]
            else:
                dims = [[2 * BL * N, 128], [N, BL], [0, N], [1, N]]
            return _ap(xt_sb[:], base, dims)

        def pair_view(tl):
            return _ap(tl[:], 0, [[FREE, 128], [E, BL], [N, N], [1, N]])

        # chunked over (b, i-half) so the engine chain pipelines
        LFREE = BL * 2 * N * TAU  # 9600
        IH = N // 2               # 24 i's per chunk
        CH = IH * N               # 1152 free elems per chunk
        for b in range(BL):
            for ih in range(2):
                i0 = ih * IH
                off = b * E + i0 * N
                pv = lambda tl: _ap(tl[:], off, [[FREE, 128], [N, IH], [1, N]])
                fl = lambda tl: _ap(tl[:], off, [[FREE, 128], [1, CH]])
                cb = lambda c, vi: _ap(
                    xt_sb[:], c * (BL * N) + b * N + (i0 if vi else 0),
                    [[2 * BL * N, 128], [1, IH], [0, N]] if vi
                    else [[2 * BL * N, 128], [0, IH], [1, N]],
                )
                nc.vector.tensor_tensor(pv(dx), cb(0, True), cb(0, False), ALU.subtract)
                nc.vector.tensor_tensor(pv(dy), cb(1, True), cb(1, False), ALU.subtract)
                nc.scalar.square(fl(dx2), fl(dx))
                nc.scalar.square(fl(dy2), fl(dy))
                nc.gpsimd.tensor_tensor(fl(dx), fl(dx2), fl(dy2), ALU.add)
                nc.scalar.sqrt(fl(dy), fl(dx))
                bm_ap = _ap(bm_sb[:], i0 * N, [[E, 128], [1, CH]])
                nc.vector.scalar_tensor_tensor(
                    fl(dx2), fl(dy), EPS, bm_ap, ALU.add, ALU.add
                )
                nc.vector.reciprocal_approx_fast(out=fl(dy2), in_=fl(dx2))
                nc.scalar.copy(fl(adj16), fl(dy2))

                for s in range(2):
                    for gl in range(IH // GI):
                        g = i0 // GI + gl
                        pt = tp_pool.tile([48, GI * TAU], F16, tag="tp")
                        for ii in range(GI):
                            i = g * GI + ii
                            src = adj16[s * 64 : s * 64 + TAU,
                                        b * E + i * N : b * E + i * N + N]
                            nc.tensor.transpose(
                                pt[:, ii * TAU : (ii + 1) * TAU], src,
                                id_sb[s * 64 : s * 64 + TAU, :]
                            )
                        dst = _ap(
                            lhsT[:],
                            (s * 64) * LFREE + b * (2 * N * TAU) + s * (N * TAU)
                            + g * GI * TAU,
                            [[LFREE, 48], [TAU, GI], [1, TAU]],
                        )
                        csrc = _ap(pt[:], 0, [[GI * TAU, 48], [TAU, GI], [1, TAU]])
                        if g % 2 == 0:
                            nc.vector.tensor_copy(dst, csrc)
                        else:
                            nc.scalar.copy(dst, csrc)

        # ---- packed matmuls + store -------------------------------------
        HF = BL * TAU * H  # hid_sb free size (12800)
        groups = [(tg * 4, min(4, TAU - tg * 4)) for tg in range((TAU + 3) // 4)]
        for b in range(BL):
            for t0, tlen in groups:
                mt = mm_pool.tile([96, 4 * H], F32, tag="mm")
                for k in range(tlen):
                    tau = t0 + k
                    w_ap = _ap(
                        lhsT[:], b * (2 * N * TAU) + tau, [[LFREE, 128], [TAU, 96]]
                    )
                    r_ap = _ap(
                        hid_sb[:], b * (TAU * H) + tau * H, [[HF, 128], [1, H]]
                    )
                    nc.tensor.matmul(
                        mt[:, k * H : (k + 1) * H], w_ap, r_ap,
                        start=True, stop=True,
                    )
                ot = ot_pool.tile([96, 4 * H], F16, tag="ot")
                if t0 % 8 == 0:
                    nc.scalar.copy(ot[:, : tlen * H], mt[:, : tlen * H])
                else:
                    nc.vector.tensor_copy(ot[:, : tlen * H], mt[:, : tlen * H])
                dst = out[b, :, :, t0 : t0 + tlen, :].rearrange(
                    "s i t h -> (s i) (t h)"
                )
                nc.sync.dma_start(dst, ot[:, : tlen * H])


# ----------------------------------------------------------------------------
# Host side
# ----------------------------------------------------------------------------

try:
    import torch  # SIMD f16<->f32 convert + layout change in one pass

    _HAVE_TORCH = True
except Exception:
    _HAVE_TORCH = False


def _prep_xt_all(locs):
    """locs [16,48,100,2] f32 -> concat-ready xt [16,128,96] (c-major per core)."""
    lc = locs.reshape(NCORES, BL, N, TAU, 2, 2)  # (core, bl, n, tau, s, c)
    tr = lc.transpose(0, 5, 4, 3, 1, 2)          # (core, c, s, tau, bl, n)
    xt = np.zeros((NCORES, 2, 2, 64, BL * N), dtype=np.float32)
    xt[:, :, :, :TAU] = tr.reshape(NCORES, 2, 2, TAU, BL * N)
    # filler rows: spread points (x=n, y=0) so junk weights stay finite
    fill = np.tile(np.arange(N, dtype=np.float32), BL)
    xt[:, 0, :, TAU:] = fill[None, None, None, :]
    return xt.reshape(NCORES * 2, 128, BL * N)


def _prep_hid_all(hidden):
    """hidden [16,48,100,128] f32 -> [16,2,48,50,128] f16 (b, s, j, tau, h)."""
    out = np.empty((B, 2, N, TAU, H), dtype=np.float16)
    if _HAVE_TORCH:
        src = torch.from_numpy(hidden).view(B, N, TAU, 2, H).permute(0, 3, 1, 2, 4)
        torch.from_numpy(out).copy_(src)
    else:
        hc = hidden.astype(np.float16).reshape(B, N, TAU, 2, H)
        out[:] = hc.transpose(0, 3, 1, 2, 4)
    return out


def _post_out_all(parts):
    """8 shards [2,2,48,50,128] f16 (b,s,i,tau,h) -> [16,48,100,128] f32."""
    res = np.empty((B, N, T, H), dtype=np.float32)
    if _HAVE_TORCH:
        dst = torch.from_numpy(res).view(B, N, TAU, 2, H)
        for k, p in parts:
            src = torch.from_numpy(p).permute(0, 2, 3, 1, 4)  # (b,i,tau,s,h)
            dst[BL * k : BL * k + BL].copy_(src)
    else:
        rv = res.reshape(B, N, TAU, 2, H)
        for k, p in parts:
            rv[BL * k : BL * k + BL] = p.astype(np.float32).transpose(0, 2, 3, 1, 4)
    return res


_BM = None
_IDENT = None


def _consts():
    global _BM, _IDENT
    if _BM is None:
        row = (BIG * np.eye(N, dtype=np.float32)).astype(np.float16).reshape(1, E)
        _BM = np.ascontiguousarray(np.repeat(row, 128, axis=0))
        idm = np.zeros((128, TAU), dtype=np.float16)
        idm[0:TAU] = np.eye(TAU, dtype=np.float16)
        idm[64 : 64 + TAU] = np.eye(TAU, dtype=np.float16)
        _IDENT = idm
    return _BM, _IDENT


LAST_EXEC_NS = None


class _FastPath:
    """Cached jit(shard_map(bass_exec)) dispatch — built once, reused per call.

    run_bass_via_pjrt (what run_bass_kernel_spmd delegates to under axon)
    rebuilds the jit wrapper on every call (~0.5s) and ships the consts and a
    zero output-donation buffer over the tunnel each time. This keeps the
    jitted callable, the replicated consts, and a device-resident donation
    buffer alive across calls.
    """

    def __init__(self, nc):
        import jax
        from jax.sharding import Mesh, PartitionSpec, NamedSharding
        from jax.experimental.shard_map import shard_map
        from concourse import bass2jax, mybir as _mybir

        self.jax = jax
        bass2jax.install_neuronx_cc_hook()
        partition_name = (
            nc.partition_id_tensor.name if nc.partition_id_tensor else None
        )
        in_names, out_names, out_avals, zero_outs = [], [], [], []
        for alloc in nc.m.functions[0].allocations:
            if not isinstance(alloc, _mybir.MemoryLocationSet):
                continue
            name = alloc.memorylocations[0].name
            if alloc.kind == "ExternalInput":
                if name != partition_name:
                    in_names.append(name)
            elif alloc.kind == "ExternalOutput":
                shape = tuple(alloc.tensor_shape)
                dtype = _mybir.dt.np(alloc.dtype)
                out_avals.append(jax.core.ShapedArray(shape, dtype))
                out_names.append(name)
                zero_outs.append(np.zeros(shape, dtype))
        n_params = len(in_names)
        in_names_all = list(in_names) + out_names
        if partition_name is not None:
            in_names_all.append(partition_name)
        self.in_names = in_names

        def _body(*args):
            operands = list(args)
            if partition_name is not None:
                operands.append(bass2jax.partition_id_tensor())
            outs = bass2jax._bass_exec_p.bind(
                *operands,
                out_avals=tuple(out_avals),
                in_names=tuple(in_names_all),
                out_names=tuple(out_names),
                lowering_input_output_aliases=(),
                sim_require_finite=True,
                sim_require_nnan=True,
                nc=nc,
            )
            return tuple(outs)

        devices = jax.devices()[:NCORES]
        self.devices = devices
        mesh = Mesh(np.asarray(devices), ("core",))
        n_outs = len(out_avals)
        self.fn = jax.jit(
            shard_map(
                _body,
                mesh=mesh,
                in_specs=(PartitionSpec("core"),) * (n_params + n_outs),
                out_specs=(PartitionSpec("core"),) * n_outs,
                check_rep=False,
            ),
            donate_argnums=tuple(range(n_params, n_params + n_outs)),
            keep_unused=True,
        )
        self.shd = NamedSharding(mesh, PartitionSpec("core"))
        bm, ident = _consts()
        self.bm_dev = jax.device_put(
            np.broadcast_to(bm, (NCORES, *bm.shape)).reshape(NCORES * 128, E),
            self.shd,
        )
        self.ident_dev = jax.device_put(
            np.broadcast_to(ident, (NCORES, *ident.shape)).reshape(NCORES * 128, TAU),
            self.shd,
        )
        # donation seed: contents never read (kernel writes every element)
        self.prev_out = jax.device_put(
            np.zeros((B, 2, N, TAU, H), np.float16), self.shd
        )
        jax.block_until_ready([self.bm_dev, self.ident_dev, self.prev_out])
        import concurrent.futures as _cf

        # 4 workers beat 8 on the single host CPU (less thrash, same overlap)
        self.pool = _cf.ThreadPoolExecutor(4)

        # AOT-compile to skip pjit python dispatch (~40ms/call)
        dummy = {
            "xt": np.zeros((NCORES * 2, 128, BL * N), np.float32),
            "hid": self._put_hid(np.zeros((B, 2, N, TAU, H), np.float16)),
            "bm": self.bm_dev,
            "ident": self.ident_dev,
        }
        self.fn_c = self.fn.lower(
            *[dummy[n] for n in self.in_names], self.prev_out
        ).compile()

    def _put_hid(self, hid_all):
        """Upload the 8 hid shards in parallel threads, assemble zero-copy."""
        jax = self.jax
        futs = [
            self.pool.submit(jax.device_put, hid_all[BL * k : BL * k + BL], d)
            for k, d in enumerate(self.devices)
        ]
        shards = [f.result() for f in futs]
        return jax.make_array_from_single_device_arrays(
            (B, 2, N, TAU, H), self.shd, shards
        )

    def _convert_put_hid(self, hidden):
        """Per-shard f32->f16 layout convert fused with async upload."""
        jax = self.jax

        def _one(k):
            sl = hidden[BL * k : BL * k + BL]
            buf = np.empty((BL, 2, N, TAU, H), dtype=np.float16)
            if _HAVE_TORCH:
                src = torch.from_numpy(sl).view(BL, N, TAU, 2, H).permute(
                    0, 3, 1, 2, 4
                )
                torch.from_numpy(buf).copy_(src)
            else:
                hc = sl.astype(np.float16).reshape(BL, N, TAU, 2, H)
                buf[:] = hc.transpose(0, 3, 1, 2, 4)
            return jax.device_put(buf, self.devices[k])

        futs = [self.pool.submit(_one, k) for k in range(NCORES)]
        shards = [f.result() for f in futs]
        return jax.make_array_from_single_device_arrays(
            (B, 2, N, TAU, H), self.shd, shards
        )

    def run(self, xt_all, hid_g, res):
        """Dispatch one full-batch execution; fill res [B,N,T,H] f32."""
        args = {
            "xt": xt_all,
            "hid": hid_g,
            "bm": self.bm_dev,
            "ident": self.ident_dev,
        }
        outs = self.fn_c(*[args[n] for n in self.in_names], self.prev_out)
        g = outs[0]

        def _fetch_one(shard):
            k = shard.index[0].start // BL
            arr = np.asarray(shard.data)  # [BL,2,N,TAU,H] (b,s,i,tau,h)
            if _HAVE_TORCH:
                dst = torch.from_numpy(res).view(B, N, TAU, 2, H)[
                    BL * k : BL * k + BL
                ]
                dst.copy_(torch.from_numpy(arr).permute(0, 2, 3, 1, 4))
            else:
                res.reshape(B, N, TAU, 2, H)[BL * k : BL * k + BL] = (
                    arr.astype(np.float32).transpose(0, 2, 3, 1, 4)
                )

        shards = sorted(g.addressable_shards, key=lambda s: s.index[0].start)
        list(self.pool.map(_fetch_one, shards))
        self.prev_out = g  # donated next call, after this call's host copies
        return res


def _prep_xt_half(locs_h):
    """locs_h [8,48,100,2] f32 -> xt [16,128,48] for the BL=1 kernel."""
    lc = locs_h.reshape(NCORES, 1, N, TAU, 2, 2)
    tr = lc.transpose(0, 5, 4, 3, 1, 2)
    xt = np.zeros((NCORES, 2, 2, 64, N), dtype=np.float32)
    xt[:, :, :, :TAU] = tr.reshape(NCORES, 2, 2, TAU, N)
    xt[:, 0, :, TAU:] = np.arange(N, dtype=np.float32)[None, None, None, :]
    return xt.reshape(NCORES * 2, 128, N)


class _HalfPath:
    """Two pipelined half-batch (BL=1) executions per call.

    Exec launch is globally gated on all shard inputs per executable, so one
    full-batch call serializes H2D -> exec -> D2H. Splitting into two 8-batch
    launches lets half B's upload/exec overlap half A's execution and D2H.
    """

    def __init__(self, st, nc1):
        import jax
        from jax.sharding import Mesh, PartitionSpec
        from jax.experimental.shard_map import shard_map
        from concourse import bass2jax, mybir as _mybir

        self.jax = jax
        self.st = st
        partition_name = (
            nc1.partition_id_tensor.name if nc1.partition_id_tensor else None
        )
        in_names, out_names, out_avals = [], [], []
        for alloc in nc1.m.functions[0].allocations:
            if not isinstance(alloc, _mybir.MemoryLocationSet):
                continue
            name = alloc.memorylocations[0].name
            if alloc.kind == "ExternalInput":
                if name != partition_name:
                    in_names.append(name)
            elif alloc.kind == "ExternalOutput":
                out_avals.append(
                    jax.core.ShapedArray(
                        tuple(alloc.tensor_shape), _mybir.dt.np(alloc.dtype)
                    )
                )
                out_names.append(name)
        n_params = len(in_names)
        in_names_all = list(in_names) + out_names
        if partition_name is not None:
            in_names_all.append(partition_name)
        self.in_names = in_names

        def _body(*args):
            operands = list(args)
            if partition_name is not None:
                operands.append(bass2jax.partition_id_tensor())
            return tuple(
                bass2jax._bass_exec_p.bind(
                    *operands,
                    out_avals=tuple(out_avals),
                    in_names=tuple(in_names_all),
                    out_names=tuple(out_names),
                    lowering_input_output_aliases=(),
                    sim_require_finite=True,
                    sim_require_nnan=True,
                    nc=nc1,
                )
            )

        mesh = Mesh(np.asarray(st.devices), ("core",))
        fn = jax.jit(
            shard_map(
                _body,
                mesh=mesh,
                in_specs=(PartitionSpec("core"),) * (n_params + 1),
                out_specs=(PartitionSpec("core"),),
                check_rep=False,
            ),
            donate_argnums=(n_params,),
            keep_unused=True,
        )
        self.bufA = jax.device_put(
            np.zeros((NCORES, 2, N, TAU, H), np.float16), st.shd
        )
        self.bufB = jax.device_put(
            np.zeros((NCORES, 2, N, TAU, H), np.float16), st.shd
        )
        jax.block_until_ready([self.bufA, self.bufB])
        dummy = {
            "xt": np.zeros((NCORES * 2, 128, N), np.float32),
            "hid": self._put_hid_half(np.zeros((NCORES, N, T, H), np.float32)),
            "bm": st.bm_dev,
            "ident": st.ident_dev,
        }
        self.fn_c = fn.lower(
            *[dummy[n] for n in in_names], self.bufA
        ).compile()

    def _put_hid_half(self, hid_h):
        """[8,48,100,128] f32 -> sharded [8,2,48,50,128] f16 (convert+upload)."""
        jax, st = self.jax, self.st

        def _one(k):
            buf = np.empty((1, 2, N, TAU, H), dtype=np.float16)
            if _HAVE_TORCH:
                src = torch.from_numpy(hid_h[k : k + 1]).view(
                    1, N, TAU, 2, H
                ).permute(0, 3, 1, 2, 4)
                torch.from_numpy(buf).copy_(src)
            else:
                hc = hid_h[k : k + 1].astype(np.float16).reshape(1, N, TAU, 2, H)
                buf[:] = hc.transpose(0, 3, 1, 2, 4)
            return jax.device_put(buf, st.devices[k])

        futs = [st.pool.submit(_one, k) for k in range(NCORES)]
        shards = [f.result() for f in futs]
        return jax.make_array_from_single_device_arrays(
            (NCORES, 2, N, TAU, H), st.shd, shards
        )

    def _fetch_half(self, g, b0, res):
        def _one(shard):
            k = shard.index[0].start
            arr = np.asarray(shard.data)  # [1,2,N,TAU,H] (b,s,i,tau,h)
            if _HAVE_TORCH:
                dst = torch.from_numpy(res).view(B, N, TAU, 2, H)[
                    b0 + k : b0 + k + 1
                ]
                dst.copy_(torch.from_numpy(arr).permute(0, 2, 3, 1, 4))
            else:
                res.reshape(B, N, TAU, 2, H)[b0 + k : b0 + k + 1] = (
                    arr.astype(np.float32).transpose(0, 2, 3, 1, 4)
                )

        shards = sorted(g.addressable_shards, key=lambda s: s.index[0].start)
        list(self.st.pool.map(_one, shards))

    def run(self, locs, hidden, res):
        st = self.st
        xtA = _prep_xt_half(locs[:NCORES])
        xtB = _prep_xt_half(locs[NCORES:])
        argsA = {
            "xt": xtA,
            "hid": self._put_hid_half(hidden[:NCORES]),
            "bm": st.bm_dev,
            "ident": st.ident_dev,
        }
        gA = self.fn_c(*[argsA[n] for n in self.in_names], self.bufA)[0]
        argsB = {
            "xt": xtB,
            "hid": self._put_hid_half(hidden[NCORES:]),
            "bm": st.bm_dev,
            "ident": st.ident_dev,
        }
        gB = self.fn_c(*[argsB[n] for n in self.in_names], self.bufB)[0]
        self._fetch_half(gA, 0, res)
        self._fetch_half(gB, NCORES, res)
        self.bufA, self.bufB = gA, gB
        return res


_NC = None
_NC1 = None
_FAST = None
_HALF = None


def _get_nc():
    global _NC
    if _NC is None:
        _NC = build_nc()
    return _NC


def _spmd_call(locs, hidden):
    """Robust path: per-call run_bass_kernel_spmd (slower, no cached state)."""
    nc = _get_nc()
    xt_all = _prep_xt_all(locs)
    hid_all = _prep_hid_all(hidden)
    bm, ident = _consts()
    in_maps = [
        {
            "xt": xt_all[2 * k : 2 * k + 2],
            "hid": hid_all[BL * k : BL * k + BL],
            "bm": bm,
            "ident": ident,
        }
        for k in range(NCORES)
    ]
    res = run_bass_kernel_spmd(nc, in_maps, list(range(NCORES)), trace=False)
    global LAST_EXEC_NS
    LAST_EXEC_NS = getattr(res, "exec_time_ns", None)
    parts = [
        (k, res.results[k]["out"].reshape(BL, 2, N, TAU, H)) for k in range(NCORES)
    ]
    return _post_out_all(parts), xt_all, hid_all


def _cold_call(locs, hidden):
    """First call: run via run_bass_kernel_spmd, then build+warm fast paths."""
    global _FAST, _HALF, _NC1
    out, xt_all, hid_all = _spmd_call(locs, hidden)
    # build + fully warm the cached dispatch so call #2 is pure fast path
    _FAST = _FastPath(_get_nc())
    _FAST.run(xt_all, _FAST._put_hid(hid_all), np.empty((B, N, T, H), np.float32))
    # _HalfPath (two pipelined BL=1 launches) measured no better than the
    # single launch once window noise was controlled for; not built by default.
    return out


def kernel(locs, hidden, rel_rec=None, rel_send=None):
    global _FAST, _HALF
    locs = np.ascontiguousarray(locs, dtype=np.float32)
    hidden = np.ascontiguousarray(hidden, dtype=np.float32)
    if _HAVE_TORCH and not hidden.flags.writeable:
        hidden = hidden.copy()
    if _FAST is None:
        return _cold_call(locs, hidden)
    res = np.empty((B, N, T, H), dtype=np.float32)
    try:
        xt_all = _prep_xt_all(locs)
        hid_g = _FAST._convert_put_hid(hidden)
        return _FAST.run(xt_all, hid_g, res)
    except Exception:
        # fast-path state (donation chain / device session) may be broken;
        # rebuild lazily on the next call and answer via the robust path now
        _FAST = None
        return _spmd_call(locs, hidden)[0]


if __name__ == "__main__":
    # smoke test with random data against a local numpy reference
    rng = np.random.default_rng(0)
    locs = rng.standard_normal((B, N, T, 2), dtype=np.float32)
    hidden = rng.standard_normal((B, N, T, H), dtype=np.float32)
    got = kernel(locs, hidden)
    x = locs[..., 0]
    y = locs[..., 1]
    d = np.sqrt((x[:, :, None] - x[:, None]) ** 2 + (y[:, :, None] - y[:, None]) ** 2)
    w = 1.0 / (d + EPS) * (1.0 - np.eye(N)[None, :, :, None])
    want = np.einsum("bijt,bjth->bith", w.astype(np.float32), hidden)
    err = np.linalg.norm(got - want) / np.linalg.norm(want)
    print("rel err vs numpy:", err)

